# revision 17
# baseline (speedup 1.0000x reference)
"""CTRNN-MD scan kernel for Trainium2 (8 NeuronCores, batch-data-parallel).

Per core (batch shard Bc=1024), per step t:
    pre = W_ext.T @ [x_t; 1] + W_rec.T @ h_{t-1}        (PE, fp32r, PSUM accum)
    h_t = relu(pre)                                      (ACT+DVE from PSUM)
    hb  = h_t as bf16                                    (DVE cast copy)
    natp = hb.T blocks via PE identity matmuls           (16x [128,128] bf16 MMs)
    nat  = natp (PSUM) -> SBUF fp32                      (ACT+DVE copies)
    out[t] = nat                                         (HWDGE fp32 store)

Host pre-folds alpha=0.5, the gate row g=gates[sub_id], and both biases into
the weights:
    W_ext [34,256]  = [0.5*W_in.T ; 0.5*(b_in + g*b_h)]
    W_rec [256,256] = (0.5*(g[:,None]*W_h) + 0.5*I).T
so the entire step is two accumulating matmul groups + one relu.

Layout/pipelining tricks:
  - x is preloaded entirely into SBUF (one [98, 25, Bc] tile: steps <25 on
    partitions 0-33, steps >=25 on partitions 64-97; matmul requires operand
    base partitions in {0,32,64}); DRAM layout [34, T, Bc] i-major so each
    load is a 100KB-contiguous run per partition.
  - batch columns are host-permuted so the transpose's mapping
    (SBUF column j*128+p -> nat[p, j, :]) lands at DRAM row p*8+j, giving the
    store one contiguous 8KB run per partition. The device thus writes `out`
    in the ORIGINAL batch order; only h_lastT needs host-side unpermuting.
  - the transpose matmuls for step t-1 are emitted between rec(t) and
    ext(t+1) on the PE stream: they fill exactly the bubble where PE waits
    for relu(t) (pre-PSUM is single-buffered).
"""

import numpy as np
import ml_dtypes

import concourse.bass as bass
import concourse.mybir as mybir
import concourse.tile as tile
from concourse import bacc
from concourse.bass_utils import run_bass_kernel_spmd

T, B, I, H = 50, 8192, 33, 256
NCORES = 8
Bc = B // NCORES  # 1024
KA = I + 1  # augmented input (ones row folded in)

F32 = mybir.dt.float32
F32R = mybir.dt.float32r
BF16 = mybir.dt.bfloat16
RELU = mybir.ActivationFunctionType.Relu

_NC_CACHE = {}


def _perm(bc):
    # column b' = j*128+p processes original batch row p*n_bi+j
    n_bi = bc // 128
    return np.arange(bc).reshape(128, n_bi).T.ravel()


def _build(t_steps=T, bc=Bc):
    """Build the per-core Bass module (SPMD: same NEFF, per-core input slices)."""
    n_ni = bc // 512  # matmul N-tiles (PSUM banks per m-tile)
    n_bi = bc // 128  # 128-batch blocks
    th = (t_steps + 1) // 2  # steps held in the low-partition x half

    nc = bacc.Bacc("TRN2", target_bir_lowering=False)
    xT_d = nc.dram_tensor("xT", [KA, t_steps, bc], F32R, kind="ExternalInput")
    wext_d = nc.dram_tensor("w_ext", [KA, H], F32R, kind="ExternalInput")
    wrec_d = nc.dram_tensor("w_rec", [H, H], F32R, kind="ExternalInput")
    ident_d = nc.dram_tensor("ident", [128, 128], BF16, kind="ExternalInput")
    out_d = nc.dram_tensor("out", [t_steps, bc, H], F32, kind="ExternalOutput")
    hlast_d = nc.dram_tensor("h_lastT", [H, bc], F32R, kind="ExternalOutput")

    with tile.TileContext(nc) as tc:
        with (
            tc.tile_pool(name="wpool", bufs=1) as wpool,
            tc.tile_pool(name="xpool", bufs=1) as xpool,
            tc.tile_pool(name="state", bufs=3) as spool,
            tc.tile_pool(name="bfpool", bufs=4) as bpool,
            tc.tile_pool(name="natpool", bufs=4) as npool,
            tc.tile_pool(name="psum", bufs=3, space="PSUM") as psump,
            tc.tile_pool(name="psumt", bufs=1, space="PSUM") as psumtp,
        ):
            # whole x resident in SBUF: steps < th at partitions 0..33,
            # steps >= th at partitions 64..97. First few steps go into
            # their own small tile (loaded first) so step 0 starts early.
            t0n = min(8, th)
            x0 = xpool.tile([KA, t0n, bc], F32R)
            nc.scalar.dma_start(x0[:], xT_d[:, :t0n, :])
            # W_ext duplicated at base partitions 0 and 64 (to pair with the
            # two x halves); W_rec split into its two 128-row k-tiles.
            wboth = wpool.tile([64 + KA, H], F32R)
            nc.scalar.dma_start(wboth[:KA], wext_d[:])
            nc.scalar.dma_start(wboth[64 : 64 + KA], wext_d[:])
            wrec_t = wpool.tile([128, 2, H], F32R)
            nc.scalar.dma_start(
                wrec_t[:], wrec_d.rearrange("(kt k) m -> k kt m", k=128)
            )
            ident = wpool.tile([128, 128], BF16)
            nc.scalar.dma_start(ident[:], ident_d[:])

            # bulk x chunks alternate between the two HWDGE rings (they
            # execute FIFO-serially per ring; sync is idle at kernel start)
            xall = xpool.tile([64 + KA, th, bc], F32R)
            if th > t0n:
                mid = (t0n + th) // 2
                nc.sync.dma_start(xall[:KA, t0n:mid, :], xT_d[:, t0n:mid, :])
                nc.scalar.dma_start(xall[:KA, mid:, :], xT_d[:, mid:th, :])
            if t_steps > th:
                mid2 = (th + t_steps) // 2
                nc.sync.dma_start(
                    xall[64 : 64 + KA, : mid2 - th, :], xT_d[:, th:mid2, :]
                )
                nc.scalar.dma_start(
                    xall[64 : 64 + KA, mid2 - th : t_steps - th, :],
                    xT_d[:, mid2:, :],
                )

            def flush_store(tp, hb_p):
                """Transpose step tp's bf16 state on the PE and store it.

                Two rounds through a 2-bank PSUM staging tile; copies of
                round k overlap the transposes of round k+1."""
                nat = npool.tile([128, n_bi, H], F32)
                for rnd in range(n_bi // 4):
                    natp = psumtp.tile([128, 2, 512], F32, tag="natp")
                    for hi in range(2):
                        for bj in range(4):
                            bi = rnd * 4 + bj
                            nc.tensor.matmul(
                                natp[
                                    :,
                                    bj // 2,
                                    (bj % 2) * 256 + hi * 128 : (bj % 2) * 256
                                    + hi * 128
                                    + 128,
                                ],
                                hb_p[:, hi, bi * 128 : (bi + 1) * 128],
                                ident[:],
                                start=True,
                                stop=True,
                            )
                    for k in range(2):
                        dst = nat[:, rnd * 4 + 2 * k : rnd * 4 + 2 * k + 2, :]
                        if k == 0:
                            nc.scalar.copy(dst, natp[:, k, :])
                        else:
                            nc.vector.tensor_copy(dst, natp[:, k, :])
                # batch-permuted store: nat[p, j, :] -> DRAM row p*n_bi+j, so
                # each partition writes one contiguous n_bi*1KB run
                nc.sync.dma_start(
                    out_d[tp].rearrange("(p j) h -> p j h", j=n_bi), nat[:]
                )

            h_prev = None
            hb_prev = None
            for t in range(t_steps):
                base = 0 if t < th else 64
                if t < min(8, th):
                    xt = x0[:, t, :]
                else:
                    xt = xall[base : base + KA, t if t < th else t - th, :]
                wext_t = wboth[base : base + KA, :]

                # per-mi PSUM tiles from a 3-deep pool: a step's matmuls
                # WAR against step t-2's relus instead of step t-1's, so
                # ext(t) never waits on relu(t-1)
                psum0 = psump.tile([128, n_ni, 512], F32, tag="pre")
                psum1 = psump.tile([128, n_ni, 512], F32, tag="pre")
                psum = [psum0, psum1]
                for mi in range(2):
                    lhs_ext = wext_t[:, mi * 128 : (mi + 1) * 128]
                    for ni in range(n_ni):
                        nc.tensor.matmul(
                            psum[mi][:, ni],
                            lhs_ext,
                            xt[:, ni * 512 : (ni + 1) * 512],
                            start=True,
                            stop=(t == 0),
                        )
                if t > 0:
                    # consume h_prev halves in the order their relus complete
                    # (per-bank relus below): (kt0,ni0), (kt1,ni0), (kt0,ni1),
                    # (kt1,ni1) -- shortens the relu->rec critical path
                    for ni in range(n_ni):
                        for kt in range(2):
                            for mi in range(2):
                                lhs_rec = wrec_t[:, kt, mi * 128 : (mi + 1) * 128]
                                nc.tensor.matmul(
                                    psum[mi][:, ni],
                                    lhs_rec,
                                    h_prev[:, kt, ni * 512 : (ni + 1) * 512],
                                    start=False,
                                    stop=(kt == 1),
                                )
                h_new = spool.tile([128, 2, bc], F32R)
                hb = bpool.tile([128, 2, bc], BF16)
                # relus split across ACT (mi=0) and DVE (mi=1) on different
                # PSUM banks (same-bank concurrent ACT+DVE access is illegal).
                # Emitted BEFORE the previous step's flush so ACT/DVE run the
                # recurrence-critical relu ahead of the store-path copies.
                for ni in range(n_ni):
                    sl = slice(ni * 512, (ni + 1) * 512)
                    nc.scalar.activation(h_new[:, 0, sl], psum[0][:, ni], RELU)
                    nc.vector.tensor_scalar_max(h_new[:, 1, sl], psum[1][:, ni], 0.0)
                for mi in range(2):
                    # bf16 copy: transpose-matmul input for the store path
                    nc.vector.tensor_copy(hb[:, mi, :], h_new[:, mi, :])

                if t > 0:
                    # PE bubble filler: transpose+store of the previous step
                    # (its hb is ready; runs while relu(t) completes)
                    flush_store(t - 1, hb_prev)

                if t == t_steps - 1:
                    nc.sync.dma_start(
                        hlast_d.rearrange("(hi p) b -> p hi b", p=128), h_new[:]
                    )
                h_prev, hb_prev = h_new, hb

            flush_store(t_steps - 1, hb_prev)

    nc.compile()
    return nc


def _prep_host(x, sub_id, gates, W_in, b_in, W_h, b_h):
    sid = int(np.asarray(sub_id))
    g = np.asarray(gates, np.float32)[sid]  # (H,)
    W_in = np.asarray(W_in, np.float32)
    W_h = np.asarray(W_h, np.float32)
    b_in = np.asarray(b_in, np.float32)
    b_h = np.asarray(b_h, np.float32)
    x = np.asarray(x, np.float32)

    c = 0.5 * (b_in + g * b_h)  # (H,)
    w_ext = np.empty((KA, H), np.float32)
    w_ext[:I] = 0.5 * W_in.T
    w_ext[I] = c
    w_rec = np.ascontiguousarray(
        (0.5 * (g[:, None] * W_h) + 0.5 * np.eye(H, dtype=np.float32)).T
    )
    ident = np.eye(128, dtype=np.float32).astype(ml_dtypes.bfloat16)

    perm = _perm(Bc)
    in_maps = []
    for ci in range(NCORES):
        # device layout [KA, T, Bc], columns permuted so the store lands in
        # original batch order
        xs = x[:, ci * Bc : (ci + 1) * Bc, :][:, perm, :]  # (T, Bc, I)
        xa = np.empty((KA, T, Bc), np.float32)
        xa[:I] = xs.transpose(2, 0, 1)
        xa[I] = 1.0
        in_maps.append(
            {"xT": xa, "w_ext": w_ext, "w_rec": w_rec, "ident": ident}
        )
    return in_maps


def _install_ntff_hook():
    """Best-effort registration of the axon NTFF profiling hook (trace runs)."""
    try:
        import sys
        import types

        if "antenv.axon_hooks" not in sys.modules:
            mod = types.ModuleType("antenv.axon_hooks")
            hook_cell = [None]
            mod.set_axon_ntff_profile_hook = lambda h: hook_cell.__setitem__(0, h)
            mod.get_axon_ntff_profile_hook = lambda: hook_cell[0]
            sys.modules["antenv.axon_hooks"] = mod
            import antenv

            antenv.axon_hooks = mod
        import antenv.axon_hooks as ah

        if ah.get_axon_ntff_profile_hook() is None:
            from trn_agent_boot.trn_boot import _ntff_profile_via_ctypes

            ah.set_axon_ntff_profile_hook(
                _ntff_profile_via_ctypes("/opt/axon/libaxon_pjrt.so")
            )
        return ah.get_axon_ntff_profile_hook() is not None
    except Exception:
        return False


def kernel_impl(inputs, trace=False):
    if "nc" not in _NC_CACHE:
        _NC_CACHE["nc"] = _build()
    nc = _NC_CACHE["nc"]

    in_maps = _prep_host(**inputs)
    if trace:
        _install_ntff_hook()
    res = run_bass_kernel_spmd(
        nc, in_maps, core_ids=list(range(NCORES)), trace=trace
    )
    out = np.concatenate([r["out"] for r in res.results], axis=1)  # (T, B, H)
    perm = _perm(Bc)
    h_parts = []
    for r in res.results:
        hl = np.empty((Bc, H), np.float32)
        hl[perm] = r["h_lastT"].T  # device column b' -> original row perm[b']
        h_parts.append(hl)
    h_last = np.ascontiguousarray(np.concatenate(h_parts, axis=0))  # (B, H)
    return (out, h_last), res


def kernel(**inputs):
    (out, h_last), _ = kernel_impl(inputs, trace=False)
    return out, h_last


# revision 18
# speedup vs baseline: 1.0589x; 1.0589x over previous
"""CTRNN-MD scan kernel for Trainium2 (8 NeuronCores, batch-data-parallel).

Per core (batch shard Bc=1024), per step t:
    pre = W_ext.T @ [x_t; 1] + W_rec.T @ h_{t-1}        (PE, fp32r, PSUM accum)
    h_t = relu(pre)                                      (ACT+DVE from PSUM)
    hb  = h_t as bf16                                    (DVE cast copy)
    natp = hb.T blocks via PE identity matmuls           (16x [128,128] bf16 MMs)
    nat  = natp (PSUM) -> SBUF fp32                      (ACT+DVE copies)
    out[t] = nat                                         (HWDGE fp32 store)

Host pre-folds alpha=0.5, the gate row g=gates[sub_id], and both biases into
the weights:
    W_ext [34,256]  = [0.5*W_in.T ; 0.5*(b_in + g*b_h)]
    W_rec [256,256] = (0.5*(g[:,None]*W_h) + 0.5*I).T
so the entire step is two accumulating matmul groups + one relu.

Layout/pipelining tricks:
  - x is preloaded entirely into SBUF (one [98, 25, Bc] tile: steps <25 on
    partitions 0-33, steps >=25 on partitions 64-97; matmul requires operand
    base partitions in {0,32,64}); DRAM layout [34, T, Bc] i-major so each
    load is a 100KB-contiguous run per partition.
  - batch columns are host-permuted so the transpose's mapping
    (SBUF column j*128+p -> nat[p, j, :]) lands at DRAM row p*8+j, giving the
    store one contiguous 8KB run per partition. The device thus writes `out`
    in the ORIGINAL batch order; only h_lastT needs host-side unpermuting.
  - the transpose matmuls for step t-1 are emitted between rec(t) and
    ext(t+1) on the PE stream: they fill exactly the bubble where PE waits
    for relu(t) (pre-PSUM is single-buffered).
"""

import numpy as np
import ml_dtypes

import concourse.bass as bass
import concourse.mybir as mybir
import concourse.tile as tile
from concourse import bacc
from concourse.bass_utils import run_bass_kernel_spmd

T, B, I, H = 50, 8192, 33, 256
NCORES = 8
Bc = B // NCORES  # 1024
KA = I + 1  # augmented input (ones row folded in)

F32 = mybir.dt.float32
F32R = mybir.dt.float32r
BF16 = mybir.dt.bfloat16
RELU = mybir.ActivationFunctionType.Relu

_NC_CACHE = {}


def _perm(bc):
    # column b' = j*128+p processes original batch row p*n_bi+j
    n_bi = bc // 128
    return np.arange(bc).reshape(128, n_bi).T.ravel()


def _build(t_steps=T, bc=Bc):
    """Build the per-core Bass module (SPMD: same NEFF, per-core input slices)."""
    n_ni = bc // 512  # matmul N-tiles (PSUM banks per m-tile)
    n_bi = bc // 128  # 128-batch blocks
    th = (t_steps + 1) // 2  # steps held in the low-partition x half

    nc = bacc.Bacc("TRN2", target_bir_lowering=False)
    xT_d = nc.dram_tensor("xT", [KA, t_steps, bc], F32R, kind="ExternalInput")
    wext_d = nc.dram_tensor("w_ext", [KA, H], F32R, kind="ExternalInput")
    wrec_d = nc.dram_tensor("w_rec", [H, H], F32R, kind="ExternalInput")
    ident_d = nc.dram_tensor("ident", [128, 128], BF16, kind="ExternalInput")
    out_d = nc.dram_tensor("out", [t_steps, bc, H], F32, kind="ExternalOutput")
    hlast_d = nc.dram_tensor("h_lastT", [H, bc], F32R, kind="ExternalOutput")

    with tile.TileContext(nc) as tc:
        with (
            tc.tile_pool(name="wpool", bufs=1) as wpool,
            tc.tile_pool(name="xpool", bufs=1) as xpool,
            tc.tile_pool(name="state", bufs=3) as spool,
            tc.tile_pool(name="bfpool", bufs=4) as bpool,
            tc.tile_pool(name="natpool", bufs=4) as npool,
            tc.tile_pool(name="psum", bufs=3, space="PSUM") as psump,
            tc.tile_pool(name="psumt", bufs=1, space="PSUM") as psumtp,
        ):
            # whole x resident in SBUF: steps < th at partitions 0..33,
            # steps >= th at partitions 64..97. First few steps go into
            # their own small tile (loaded first) so step 0 starts early.
            t0n = min(8, th)
            x0 = xpool.tile([KA, t0n, bc], F32R)
            nc.scalar.dma_start(x0[:], xT_d[:, :t0n, :])
            # W_ext duplicated at base partitions 0 and 64 (to pair with the
            # two x halves); W_rec split into its two 128-row k-tiles.
            wboth = wpool.tile([64 + KA, H], F32R)
            nc.scalar.dma_start(wboth[:KA], wext_d[:])
            nc.scalar.dma_start(wboth[64 : 64 + KA], wext_d[:])
            wrec_t = wpool.tile([128, 2, H], F32R)
            nc.scalar.dma_start(
                wrec_t[:], wrec_d.rearrange("(kt k) m -> k kt m", k=128)
            )
            ident = wpool.tile([128, 128], BF16)
            nc.scalar.dma_start(ident[:], ident_d[:])

            # bulk x chunks go via SWDGE (gpsimd) -- its own DMA queue, so
            # they don't serialize behind either HWDGE ring (stores/misc)
            xall = xpool.tile([64 + KA, th, bc], F32R)
            if th > t0n:
                mid = (t0n + th) // 2
                nc.gpsimd.dma_start(xall[:KA, t0n:mid, :], xT_d[:, t0n:mid, :])
                nc.gpsimd.dma_start(xall[:KA, mid:, :], xT_d[:, mid:th, :])
            if t_steps > th:
                mid2 = (th + t_steps) // 2
                nc.gpsimd.dma_start(
                    xall[64 : 64 + KA, : mid2 - th, :], xT_d[:, th:mid2, :]
                )
                nc.gpsimd.dma_start(
                    xall[64 : 64 + KA, mid2 - th : t_steps - th, :],
                    xT_d[:, mid2:, :],
                )

            def flush_store(tp, hb_p):
                """Transpose step tp's bf16 state on the PE and store it.

                Two rounds through a 2-bank PSUM staging tile; copies of
                round k overlap the transposes of round k+1."""
                nat = npool.tile([128, n_bi, H], F32)
                for rnd in range(n_bi // 4):
                    natp = psumtp.tile([128, 2, 512], F32, tag="natp")
                    for hi in range(2):
                        for bj in range(4):
                            bi = rnd * 4 + bj
                            nc.tensor.matmul(
                                natp[
                                    :,
                                    bj // 2,
                                    (bj % 2) * 256 + hi * 128 : (bj % 2) * 256
                                    + hi * 128
                                    + 128,
                                ],
                                hb_p[:, hi, bi * 128 : (bi + 1) * 128],
                                ident[:],
                                start=True,
                                stop=True,
                            )
                    for k in range(2):
                        dst = nat[:, rnd * 4 + 2 * k : rnd * 4 + 2 * k + 2, :]
                        if k == 0:
                            nc.scalar.copy(dst, natp[:, k, :])
                        else:
                            nc.vector.tensor_copy(dst, natp[:, k, :])
                # batch-permuted store: nat[p, j, :] -> DRAM row p*n_bi+j, so
                # each partition writes one contiguous n_bi*1KB run
                nc.sync.dma_start(
                    out_d[tp].rearrange("(p j) h -> p j h", j=n_bi), nat[:]
                )

            h_prev = None
            hb_prev = None
            for t in range(t_steps):
                base = 0 if t < th else 64
                if t < min(8, th):
                    xt = x0[:, t, :]
                else:
                    xt = xall[base : base + KA, t if t < th else t - th, :]
                wext_t = wboth[base : base + KA, :]

                # per-mi PSUM tiles from a 3-deep pool: a step's matmuls
                # WAR against step t-2's relus instead of step t-1's, so
                # ext(t) never waits on relu(t-1)
                psum0 = psump.tile([128, n_ni, 512], F32, tag="pre")
                psum1 = psump.tile([128, n_ni, 512], F32, tag="pre")
                psum = [psum0, psum1]
                for mi in range(2):
                    lhs_ext = wext_t[:, mi * 128 : (mi + 1) * 128]
                    for ni in range(n_ni):
                        nc.tensor.matmul(
                            psum[mi][:, ni],
                            lhs_ext,
                            xt[:, ni * 512 : (ni + 1) * 512],
                            start=True,
                            stop=(t == 0),
                        )
                if t > 0:
                    # consume h_prev halves in the order their relus complete
                    # (per-bank relus below): (kt0,ni0), (kt1,ni0), (kt0,ni1),
                    # (kt1,ni1) -- shortens the relu->rec critical path
                    for ni in range(n_ni):
                        for kt in range(2):
                            for mi in range(2):
                                lhs_rec = wrec_t[:, kt, mi * 128 : (mi + 1) * 128]
                                nc.tensor.matmul(
                                    psum[mi][:, ni],
                                    lhs_rec,
                                    h_prev[:, kt, ni * 512 : (ni + 1) * 512],
                                    start=False,
                                    stop=(kt == 1),
                                )
                h_new = spool.tile([128, 2, bc], F32R)
                hb = bpool.tile([128, 2, bc], BF16)
                # relus split across ACT (mi=0) and DVE (mi=1) on different
                # PSUM banks (same-bank concurrent ACT+DVE access is illegal).
                # Emitted BEFORE the previous step's flush so ACT/DVE run the
                # recurrence-critical relu ahead of the store-path copies.
                for ni in range(n_ni):
                    sl = slice(ni * 512, (ni + 1) * 512)
                    nc.scalar.activation(h_new[:, 0, sl], psum[0][:, ni], RELU)
                    nc.vector.tensor_scalar_max(h_new[:, 1, sl], psum[1][:, ni], 0.0)
                for mi in range(2):
                    # bf16 copy: transpose-matmul input for the store path
                    nc.vector.tensor_copy(hb[:, mi, :], h_new[:, mi, :])

                if t > 0:
                    # PE bubble filler: transpose+store of the previous step
                    # (its hb is ready; runs while relu(t) completes)
                    flush_store(t - 1, hb_prev)

                if t == t_steps - 1:
                    nc.sync.dma_start(
                        hlast_d.rearrange("(hi p) b -> p hi b", p=128), h_new[:]
                    )
                h_prev, hb_prev = h_new, hb

            flush_store(t_steps - 1, hb_prev)

    nc.compile()
    return nc


def _prep_host(x, sub_id, gates, W_in, b_in, W_h, b_h):
    sid = int(np.asarray(sub_id))
    g = np.asarray(gates, np.float32)[sid]  # (H,)
    W_in = np.asarray(W_in, np.float32)
    W_h = np.asarray(W_h, np.float32)
    b_in = np.asarray(b_in, np.float32)
    b_h = np.asarray(b_h, np.float32)
    x = np.asarray(x, np.float32)

    c = 0.5 * (b_in + g * b_h)  # (H,)
    w_ext = np.empty((KA, H), np.float32)
    w_ext[:I] = 0.5 * W_in.T
    w_ext[I] = c
    w_rec = np.ascontiguousarray(
        (0.5 * (g[:, None] * W_h) + 0.5 * np.eye(H, dtype=np.float32)).T
    )
    ident = np.eye(128, dtype=np.float32).astype(ml_dtypes.bfloat16)

    perm = _perm(Bc)
    in_maps = []
    for ci in range(NCORES):
        # device layout [KA, T, Bc], columns permuted so the store lands in
        # original batch order
        xs = x[:, ci * Bc : (ci + 1) * Bc, :][:, perm, :]  # (T, Bc, I)
        xa = np.empty((KA, T, Bc), np.float32)
        xa[:I] = xs.transpose(2, 0, 1)
        xa[I] = 1.0
        in_maps.append(
            {"xT": xa, "w_ext": w_ext, "w_rec": w_rec, "ident": ident}
        )
    return in_maps


def _install_ntff_hook():
    """Best-effort registration of the axon NTFF profiling hook (trace runs)."""
    try:
        import sys
        import types

        if "antenv.axon_hooks" not in sys.modules:
            mod = types.ModuleType("antenv.axon_hooks")
            hook_cell = [None]
            mod.set_axon_ntff_profile_hook = lambda h: hook_cell.__setitem__(0, h)
            mod.get_axon_ntff_profile_hook = lambda: hook_cell[0]
            sys.modules["antenv.axon_hooks"] = mod
            import antenv

            antenv.axon_hooks = mod
        import antenv.axon_hooks as ah

        if ah.get_axon_ntff_profile_hook() is None:
            from trn_agent_boot.trn_boot import _ntff_profile_via_ctypes

            ah.set_axon_ntff_profile_hook(
                _ntff_profile_via_ctypes("/opt/axon/libaxon_pjrt.so")
            )
        return ah.get_axon_ntff_profile_hook() is not None
    except Exception:
        return False


def kernel_impl(inputs, trace=False):
    if "nc" not in _NC_CACHE:
        _NC_CACHE["nc"] = _build()
    nc = _NC_CACHE["nc"]

    in_maps = _prep_host(**inputs)
    if trace:
        _install_ntff_hook()
    res = run_bass_kernel_spmd(
        nc, in_maps, core_ids=list(range(NCORES)), trace=trace
    )
    out = np.concatenate([r["out"] for r in res.results], axis=1)  # (T, B, H)
    perm = _perm(Bc)
    h_parts = []
    for r in res.results:
        hl = np.empty((Bc, H), np.float32)
        hl[perm] = r["h_lastT"].T  # device column b' -> original row perm[b']
        h_parts.append(hl)
    h_last = np.ascontiguousarray(np.concatenate(h_parts, axis=0))  # (B, H)
    return (out, h_last), res


def kernel(**inputs):
    (out, h_last), _ = kernel_impl(inputs, trace=False)
    return out, h_last
